# revision 5
# baseline (speedup 1.0000x reference)
"""Trainium2 Bass kernel for nn_Compute_fg_bg_similarity.

Computes, for each pixel and each of its 8 neighbors (3x3 window minus center,
zero padding at image borders):
  W_ij      = exp(-||lab_p - lab_q||_2)          (8 maps)
  vector_ij = ||vec_p - vec_q||_2                (8 maps)
  sim_maps  = concat([W_ij, vector_ij], axis=1)  -> (B, 16, H, W)
  loss      = mean((W_ij >= 0.1) * vector_ij)

Sharding: pure data parallel over B=16 across 8 cores (2 images/core).
Loss partials are reduced on host.
"""

import numpy as np
from contextlib import ExitStack

import concourse.bass as bass
import concourse.tile as tile
from concourse import mybir
from concourse.bass_utils import run_bass_kernel_spmd

N_CORES = 8
B_FULL = 16
B_LOCAL = B_FULL // N_CORES
H = 256
W = 256
C_LAB = 3
C_VEC = 6
THRESH = 0.1

# Neighbor offsets in reference order: [(i,j) for i in 0..2 for j in 0..2],
# center removed, as (oy, ox) relative offsets.
K_OFFS = [(-1, -1), (-1, 0), (-1, 1), (0, -1), (0, 1), (1, -1), (1, 0), (1, 1)]

F32 = mybir.dt.float32
AX = mybir.AxisListType.X
OP = mybir.AluOpType
AF = mybir.ActivationFunctionType


def _build_kernel(ctx: ExitStack, tc: "tile.TileContext", lab, vec, maps_out, loss_out):
    nc = tc.nc

    persist = ctx.enter_context(tc.tile_pool(name="persist", bufs=1))
    scratch = ctx.enter_context(tc.tile_pool(name="scratch", bufs=2))
    outp = ctx.enter_context(tc.tile_pool(name="outp", bufs=4))

    # Channel-stacked, zero-padded (cols 0 and 257) input tiles.
    # [128 part(row in chunk), C, chunk, 258]; three row-shift variants.
    VARS = ("up", "mid", "dn")
    lab_t = {v: persist.tile([128, C_LAB, 2, 258], F32, tag=f"lab_{v}", name=f"lab_{v}") for v in VARS}
    vec_t = {v: persist.tile([128, C_VEC, 2, 258], F32, tag=f"vec_{v}", name=f"vec_{v}") for v in VARS}
    acc = persist.tile([128, B_LOCAL * 8], F32, tag="acc")

    # One-time zeroing (persistent slots; DMA loads only overwrite interiors).
    for t in list(lab_t.values()) + list(vec_t.values()):
        nc.vector.memset(t[:], 0.0)

    for b in range(B_LOCAL):
        # ---- load 3 row-shift variants of all channels ----
        for src, tset, C in ((lab, lab_t, C_LAB), (vec, vec_t, C_VEC)):
            for c in range(C):
                img = src[b, c]  # [256, 256]
                nc.sync.dma_start(
                    out=tset["mid"][:, c, :, 1:257],
                    in_=img.rearrange("(k p) w -> p k w", p=128),
                )
                # up: row index = chunk*128 + p - 1
                nc.sync.dma_start(out=tset["up"][1:128, c, 0, 1:257], in_=img[0:127, :])
                nc.sync.dma_start(out=tset["up"][:, c, 1, 1:257], in_=img[127:255, :])
                # dn: row index = chunk*128 + p + 1
                nc.sync.dma_start(out=tset["dn"][:, c, 0, 1:257], in_=img[1:129, :])
                nc.sync.dma_start(out=tset["dn"][0:127, c, 1, 1:257], in_=img[129:256, :])

        vmap_of = {-1: "up", 0: "mid", 1: "dn"}
        for j, (oy, ox) in enumerate(K_OFFS):
            vl = lab_t[vmap_of[oy]]
            vv = vec_t[vmap_of[oy]]
            c0, c1 = 1 + ox, 257 + ox

            # lab squared diff sum over channels
            dl = scratch.tile([128, C_LAB, 2, 256], F32, tag="dl")
            nc.vector.tensor_sub(dl[:], lab_t["mid"][:, :, :, 1:257], vl[:, :, :, c0:c1])
            sl = scratch.tile([128, C_LAB, 2, 256], F32, tag="sl")
            nc.vector.tensor_mul(sl[:], dl[:], dl[:])
            ul = scratch.tile([128, 2, 256], F32, tag="ul")
            nc.vector.tensor_add(ul[:], sl[:, 0], sl[:, 1])
            nc.vector.tensor_add(ul[:], ul[:], sl[:, 2])

            # vec squared diff sum over channels
            dv = scratch.tile([128, C_VEC, 2, 256], F32, tag="dv")
            nc.vector.tensor_sub(dv[:], vec_t["mid"][:, :, :, 1:257], vv[:, :, :, c0:c1])
            sv = scratch.tile([128, C_VEC, 2, 256], F32, tag="sv")
            nc.vector.tensor_mul(sv[:], dv[:], dv[:])
            pv = scratch.tile([128, 3, 2, 256], F32, tag="pv")
            nc.vector.tensor_add(pv[:], sv[:, 0:5:2], sv[:, 1:6:2])
            uv = scratch.tile([128, 2, 256], F32, tag="uv")
            nc.vector.tensor_add(uv[:], pv[:, 0], pv[:, 1])
            nc.vector.tensor_add(uv[:], uv[:], pv[:, 2])

            # W = exp(-sqrt(ul)); vec_n = sqrt(uv)
            nl = scratch.tile([128, 2, 256], F32, tag="nl")
            nc.scalar.activation(nl[:], ul[:], AF.Sqrt)
            wmap = outp.tile([128, 2, 256], F32, tag="wmap")
            nc.scalar.activation(wmap[:], nl[:], AF.Exp, scale=-1.0)
            vmap = outp.tile([128, 2, 256], F32, tag="vmap")
            nc.scalar.activation(vmap[:], uv[:], AF.Sqrt)

            # loss partial: sum((W >= 0.1) * vec_n)
            stto = scratch.tile([128, 2, 256], F32, tag="stto")
            col = b * 8 + j
            nc.vector.scalar_tensor_tensor(
                out=stto[:],
                in0=wmap[:],
                scalar=THRESH,
                in1=vmap[:],
                op0=OP.is_ge,
                op1=OP.mult,
                accum_out=acc[:, col : col + 1],
            )

            nc.sync.dma_start(
                out=maps_out[b, j].rearrange("(k p) w -> p k w", p=128), in_=wmap[:]
            )
            nc.sync.dma_start(
                out=maps_out[b, 8 + j].rearrange("(k p) w -> p k w", p=128), in_=vmap[:]
            )

    lvec = persist.tile([128, 1], F32, tag="lvec")
    nc.vector.tensor_reduce(lvec[:], acc[:], AX, OP.add)
    nc.sync.dma_start(out=loss_out[:], in_=lvec[:])


def _split_multi_waits(nc, max_waits=1):
    """Walrus's TRN2 codegen rejects instructions carrying several sync-wait
    conditions (the Tile tail drain gets 3).  Hoist extras into standalone
    single-wait NoOps placed immediately before the offending instruction."""
    n = 0
    for f in nc.m.functions:
        for bb in f.blocks:
            insts = bb.instructions
            i = 0
            while i < len(insts):
                inst = insts[i]
                si = inst.sync_info
                if si is not None and si.on_wait and len(si.on_wait) > max_waits:
                    waits = list(si.on_wait)
                    keep = waits[:max_waits]
                    extra = waits[max_waits:]
                    nops = []
                    for w in extra:
                        n += 1
                        nops.append(
                            mybir.InstNoOp(
                                name=f"{inst.name}-wsplit-{n}",
                                engine=inst.engine,
                                ins=[],
                                outs=[],
                                sync_info=mybir.SyncInfo(on_wait=[w], on_update=[]),
                            )
                        )
                    inst.sync_info = mybir.SyncInfo(
                        on_wait=keep, on_update=list(si.on_update or [])
                    )
                    insts[i:i] = nops
                    i += len(nops)
                i += 1
    return n


def build_nc():
    nc = bass.Bass()
    lab = nc.dram_tensor(
        "images_lab", [B_LOCAL, C_LAB, H, W], F32, kind="ExternalInput"
    )
    vec = nc.dram_tensor(
        "pixel_vector", [B_LOCAL, C_VEC, H, W], F32, kind="ExternalInput"
    )
    maps_out = nc.dram_tensor(
        "sim_maps", [B_LOCAL, 16, H, W], F32, kind="ExternalOutput"
    )
    loss_out = nc.dram_tensor("loss_part", [128, 1], F32, kind="ExternalOutput")
    with tile.TileContext(nc) as tc:
        with ExitStack() as ctx:
            _build_kernel(ctx, tc, lab, vec, maps_out, loss_out)
    _split_multi_waits(nc)
    return nc


_NC = None


def _get_nc():
    global _NC
    if _NC is None:
        _NC = build_nc()
    return _NC


def _make_in_maps(images_lab, pixel_vector):
    images_lab = np.ascontiguousarray(images_lab, dtype=np.float32)
    pixel_vector = np.ascontiguousarray(pixel_vector, dtype=np.float32)
    assert images_lab.shape == (B_FULL, C_LAB, H, W), images_lab.shape
    assert pixel_vector.shape == (B_FULL, C_VEC, H, W), pixel_vector.shape
    return [
        {
            "images_lab": images_lab[i * B_LOCAL : (i + 1) * B_LOCAL],
            "pixel_vector": pixel_vector[i * B_LOCAL : (i + 1) * B_LOCAL],
        }
        for i in range(N_CORES)
    ]


def _postprocess(results):
    sim_maps = np.concatenate(
        [results[i]["sim_maps"] for i in range(N_CORES)], axis=0
    )
    total = sum(
        results[i]["loss_part"].astype(np.float64).sum() for i in range(N_CORES)
    )
    loss = np.float32(total / (B_FULL * 8 * H * W))
    return sim_maps, loss


def kernel(images_lab, pixel_vector):
    nc = _get_nc()
    in_maps = _make_in_maps(images_lab, pixel_vector)
    res = run_bass_kernel_spmd(nc, in_maps, list(range(N_CORES)))
    return _postprocess(res.results)


def run_traced(images_lab, pixel_vector, **kw):
    """Like kernel() but with NTFF tracing; returns (outputs, BassKernelResults)."""
    nc = _get_nc()
    in_maps = _make_in_maps(images_lab, pixel_vector)
    res = run_bass_kernel_spmd(nc, in_maps, list(range(N_CORES)), trace=True, **kw)
    return _postprocess(res.results), res
